# revision 4
# baseline (speedup 1.0000x reference)
"""DeepSeek-MoE layer on 8 Trainium2 NeuronCores (v3: merged phases).

Sharding: expert-parallel routed experts (1 expert/core, full x replicated so
no token all-to-all is needed), tensor-parallel shared expert (I_S split 8
ways), single ReduceScatter (16 pipelined fp16 chunks) combines routed +
shared partial sums; each core holds 1/16*1/8 row chunks of the output.

v3 changes vs v2 (1.80ms -> target ~1.35ms):
  - Phase A merges the router pass with the shared-expert gate/up compute:
    the DMA-bound router window (xht/xlt/xh streams) is filled with 3
    interleaved shared-gu supertiles, the remaining 13 run while R2/R3
    (top-2 + capacity dispatch) executes on Vector/GpSimd, so the PE never
    idles during routing.  hsT supertiles spill to DRAM (cheap DMA) and are
    streamed back in P3 where the shared down-proj fuses with the
    routed-combine + ReduceScatter, exactly as v2.
  - Router fp16-split passes packed: lhsT [rwh|rwl] computes wh*xh and
    wl*xh in one matmul group half, wh*xl accumulated on rows 0:8;
    transpose [16] columns then one vector add folds the halves
    (24 -> 16 matmuls per supertile).
  - P3 gathers prefetch 2 supertiles ahead of each ReduceScatter issue so
    the first RS's inter-core skew wait cannot starve the combine; output
    cast DMAs are issued one chunk late for the same reason.
"""

import numpy as np
import ml_dtypes

import concourse.bass as bass
import concourse.mybir as mybir
import concourse.tile as tile
from concourse import bacc
from concourse.masks import make_identity
from concourse.bass_utils import run_bass_kernel_spmd

dt = mybir.dt
Alu = mybir.AluOpType
Act = mybir.ActivationFunctionType

P = 128

FULL_CFG = dict(T=8192, H=1024, E=8, IR=4096, IS=8192, CAP=1280, NC=8, NCH=16)


def build(cfg):
    T, H, E, IR, IS, CAP, NCORES = (
        cfg[k] for k in ("T", "H", "E", "IR", "IS", "CAP", "NC")
    )
    J = T // P            # 128-token tiles
    HC = H // P           # h chunks of 128
    ISH = IS // NCORES    # shared-expert intermediate shard
    CT = CAP // P         # capacity tiles of 128 slots
    NCH = cfg.get("NCH", 16)  # reduce-scatter chunks
    JCH = J // NCH        # token tiles per RS chunk
    ST = 4                # token tiles per shared-expert supertile
    STT = ST * P
    NST = J // ST
    assert J % NCH == 0 and J % ST == 0 and CAP % P == 0
    IRT = IR // P
    ISHT = ISH // P
    NHT = H // 512
    GI = 3                # shared-gu supertiles interleaved into router window
    GPOST = 2             # shared-gu supertiles between router end and R2/R3

    nc = bacc.Bacc(None)

    f32, f16, i32 = dt.float32, dt.float16, dt.int32

    xh_in = nc.declare_dram_parameter("xh", [T, H], f16, isOutput=False)
    xht_in = nc.declare_dram_parameter("xht", [H, T], f16, isOutput=False)
    xlt_in = nc.declare_dram_parameter("xlt", [H, T], f16, isOutput=False)
    rw16_in = nc.declare_dram_parameter("rw16", [H, 2 * E], f16, isOutput=False)
    rg_in = nc.declare_dram_parameter("rg", [H, IR], f16, isOutput=False)
    ru_in = nc.declare_dram_parameter("ru", [H, IR], f16, isOutput=False)
    rd_in = nc.declare_dram_parameter("rd", [IR, H], f16, isOutput=False)
    sg_in = nc.declare_dram_parameter("sg", [H, ISH], f16, isOutput=False)
    su_in = nc.declare_dram_parameter("su", [H, ISH], f16, isOutput=False)
    sd_in = nc.declare_dram_parameter("sd", [ISH, H], f16, isOutput=False)
    eidf_in = nc.declare_dram_parameter("eidf", [P, J], f32, isOutput=False)
    iotaf_in = nc.declare_dram_parameter("iotaf", [P, J * E], f32, isOutput=False)
    tokidf_in = nc.declare_dram_parameter("tokidf", [P, J], f32, isOutput=False)
    slotiota_in = nc.declare_dram_parameter("slotiota", [P, CAP // P], f32, isOutput=False)
    utri_in = nc.declare_dram_parameter("utri", [J, J], f32, isOutput=False)
    o_out = nc.declare_dram_parameter("o", [NCH, T // NCH // NCORES, H], f32,
                                      isOutput=True)

    replica_groups = [list(range(NCORES))]

    with tile.TileContext(nc) as tc:
        with tc.tile_pool(name="dram", bufs=1, space="DRAM") as drp, \
             tc.tile_pool(name="pers", bufs=1) as pers:
            vlin_dram = drp.tile([T], f32)
            dlin_dram = drp.tile([CAP], f32)
            eout_dram = drp.tile([CAP, H], f16)
            hs_dram = drp.tile([NST, P, ISHT * STT], f16)
            chunk_dram = [drp.tile([T // NCH, H], f16, name=f"partial{k}")
                          for k in range(NCH)]
            rsout_dram = [drp.tile([T // NCH // NCORES, H], f16,
                                   name=f"rsout{k}") for k in range(NCH)]

            ident = pers.tile([P, P], f32)
            make_identity(nc, ident[:])
            rw16_sb = pers.tile([P, HC, 2 * E], f16)
            nc.sync.dma_start(out=rw16_sb[:],
                              in_=rw16_in[:].rearrange("(hc hp) e -> hp hc e", hp=P))
            eidf_sb = pers.tile([P, J], f32)
            nc.sync.dma_start(out=eidf_sb[:], in_=eidf_in[:])
            iotaf_sb = pers.tile([P, J, E], f32)
            nc.sync.dma_start(out=iotaf_sb[:],
                              in_=iotaf_in[:].rearrange("p (j e) -> p j e", e=E))
            tokidf_sb = pers.tile([P, J], f32)
            nc.sync.dma_start(out=tokidf_sb[:], in_=tokidf_in[:])
            slotiota_sb = pers.tile([P, CT], f32)
            nc.sync.dma_start(out=slotiota_sb[:], in_=slotiota_in[:])
            utri_sb = pers.tile([J, J], f32)
            nc.sync.dma_start(out=utri_sb[:], in_=utri_in[:])

            eps_sb = pers.tile([P, 1], f32)
            nc.vector.memset(eps_sb[:], float(np.finfo(np.float32).eps))

            z_all = pers.tile([P, J, E], f32)
            ss_all = pers.tile([P, J], f32)
            rms_all = pers.tile([P, J], f32)
            wv_pm = pers.tile([P, J], f32)       # combine weight per token
            slotg_i32 = pers.tile([P, J], i32)   # clamped slot for gather
            disp_sb = pers.tile([P, CT], i32)    # dispatch token ids
            sdw = pers.tile([P, ISHT, H], f16)   # shared down weights (for P3)

            def shared_gu(xTs, hsT, spool, pspool):
                """gate/up + silu for one shared-expert supertile."""
                for it in range(ISHT):
                    ps_g3 = pspool.tile([P, STT], f32, tag="ps_g3", bufs=2)
                    ps_u3 = pspool.tile([P, STT], f32, tag="ps_u3", bufs=2)
                    for hc in range(HC):
                        nc.tensor.matmul(
                            out=ps_g3[:], lhsT=sgw[:, hc, it * P:(it + 1) * P],
                            rhs=xTs[:, hc, :],
                            start=(hc == 0), stop=(hc == HC - 1))
                        nc.tensor.matmul(
                            out=ps_u3[:], lhsT=suw[:, hc, it * P:(it + 1) * P],
                            rhs=xTs[:, hc, :],
                            start=(hc == 0), stop=(hc == HC - 1))
                    sil3 = spool.tile([P, STT], f32, tag="sil3", bufs=2)
                    nc.scalar.activation(out=sil3[:], in_=ps_g3[:],
                                         func=Act.Sigmoid)
                    nc.vector.tensor_tensor(out=sil3[:], in0=sil3[:],
                                            in1=ps_g3[:], op=Alu.mult)
                    nc.vector.tensor_tensor(out=hsT[:, it, :], in0=sil3[:],
                                            in1=ps_u3[:], op=Alu.mult)

            # ============ Phase A: router + shared gate/up, merged ===========
            with tc.tile_pool(name="phA", bufs=1) as phA, \
                 tc.tile_pool(name="gups", bufs=1, space="PSUM") as gups:
                sgw = phA.tile([P, HC, ISH], f16)
                nc.sync.dma_start(
                    out=sgw[:], in_=sg_in[:].rearrange("(hc hp) i -> hp hc i", hp=P))
                suw = phA.tile([P, HC, ISH], f16)
                nc.sync.dma_start(
                    out=suw[:], in_=su_in[:].rearrange("(hc hp) i -> hp hc i", hp=P))

                def load_xT(src, st, tag, bufs=3):
                    t0 = st * STT
                    xT = phA.tile([P, HC, STT], f16, tag=tag, bufs=bufs)
                    nc.sync.dma_start(
                        out=xT[:],
                        in_=src[:, t0:t0 + STT].rearrange(
                            "(hc hp) t -> hp hc t", hp=P))
                    return xT

                def gu_supertile(st, xTs):
                    hsT = phA.tile([P, ISHT, STT], f16, tag="hsT", bufs=2)
                    shared_gu(xTs, hsT, phA, gups)
                    nc.sync.dma_start(out=hs_dram[st], in_=hsT[:])

                with tc.tile_pool(name="rtps", bufs=1, space="PSUM") as rtps:
                    for st in range(NST):
                        xTh = load_xT(xht_in, st, "xTh")
                        xTl = load_xT(xlt_in, st, "xTl")
                        # router: z = wh*xh + wl*xh (packed rows) then wh*xl
                        zps = rtps.tile([2 * E, STT], f32, tag="zps", bufs=2)
                        k, n_mm = 0, 2 * HC
                        for hc in range(HC):
                            nc.tensor.matmul(out=zps[:], lhsT=rw16_sb[:, hc, :],
                                             rhs=xTh[:, hc, :],
                                             start=(k == 0), stop=False)
                            k += 1
                        for hc in range(HC):
                            nc.tensor.matmul(out=zps[:E, :],
                                             lhsT=rw16_sb[:, hc, :E],
                                             rhs=xTl[:, hc, :],
                                             start=False, stop=(k == n_mm - 1),
                                             skip_group_check=True)
                            k += 1
                        ztmp = phA.tile([2 * E, STT], f32, tag="ztmp", bufs=2)
                        nc.vector.tensor_copy(out=ztmp[:], in_=zps[:])
                        for q in range(ST):
                            ztr = rtps.tile([P, 2 * E], f32, tag="ztr", bufs=2)
                            nc.tensor.transpose(ztr[:], ztmp[:, q * P:(q + 1) * P],
                                                ident[:2 * E, :2 * E])
                            ztr_sb = phA.tile([P, 2 * E], f32, tag="ztr_sb",
                                              bufs=2)
                            nc.vector.tensor_copy(out=ztr_sb[:], in_=ztr[:])
                            nc.vector.tensor_tensor(
                                out=z_all[:, st * ST + q, :],
                                in0=ztr_sb[:, :E], in1=ztr_sb[:, E:], op=Alu.add)
                        # sum(x^2) stream for the RMS scale (scalar engine)
                        for q in range(ST):
                            j = st * ST + q
                            xr = phA.tile([P, H], f16, tag="xr", bufs=6)
                            nc.sync.dma_start(out=xr[:],
                                              in_=xh_in[j * P:(j + 1) * P, :])
                            sq = phA.tile([P, H], f32, tag="sq", bufs=2)
                            nc.scalar.activation(out=sq[:], in_=xr[:],
                                                 func=Act.Square,
                                                 accum_out=ss_all[:, j:j + 1])
                        if st < GI:
                            gu_supertile(st, xTh)
                    # bridge supertiles: cover R2's latency (xht reload)
                    for st in range(GI, GI + GPOST):
                        gu_supertile(st, load_xT(xht_in, st, "xTg"))

                # ---------------- R2: top-2 + weights -----------------------
                with tc.tile_pool(name="r2", bufs=1) as r2, \
                     tc.tile_pool(name="r2ps", bufs=1, space="PSUM") as r2ps:
                    srt = r2.tile([P, J], f32)
                    nc.scalar.activation(out=srt[:], in_=ss_all[:], func=Act.Sqrt,
                                         scale=1.0 / H, bias=eps_sb[:])
                    nc.vector.reciprocal(out=rms_all[:], in_=srt[:])

                    m1 = r2.tile([P, J], f32)
                    m2 = r2.tile([P, J], f32)
                    idx1 = r2.tile([P, J], f32)
                    idx2 = r2.tile([P, J], f32)
                    eq = r2.tile([P, J, E], f32)
                    tmpje = r2.tile([P, J, E], f32)
                    tmp = r2.tile([P, J], f32)
                    rw1 = r2.tile([P, J], f32)
                    rw2 = r2.tile([P, J], f32)

                    nc.vector.tensor_reduce(out=m1[:], in_=z_all[:],
                                            axis=mybir.AxisListType.X, op=Alu.max)
                    m1b = m1[:].rearrange("p j -> p j ()").to_broadcast([P, J, E])
                    nc.vector.tensor_tensor(out=eq[:], in0=z_all[:], in1=m1b,
                                            op=Alu.is_ge)
                    # idx1 = min over e of (eq ? iota : 9)
                    nc.vector.scalar_tensor_tensor(out=tmpje[:], in0=iotaf_sb[:],
                                                   scalar=-9.0, in1=eq[:],
                                                   op0=Alu.add, op1=Alu.mult)
                    nc.vector.tensor_scalar_add(tmpje[:], tmpje[:], 9.0)
                    nc.vector.tensor_reduce(out=idx1[:], in_=tmpje[:],
                                            axis=mybir.AxisListType.X, op=Alu.min)
                    # mask out the top-1 positions, then find second max
                    nc.vector.scalar_tensor_tensor(out=eq[:], in0=eq[:],
                                                   scalar=-1e30, in1=z_all[:],
                                                   op0=Alu.mult, op1=Alu.add)
                    nc.vector.tensor_reduce(out=m2[:], in_=eq[:],
                                            axis=mybir.AxisListType.X, op=Alu.max)
                    m2b = m2[:].rearrange("p j -> p j ()").to_broadcast([P, J, E])
                    nc.vector.tensor_tensor(out=eq[:], in0=eq[:], in1=m2b,
                                            op=Alu.is_ge)
                    nc.vector.scalar_tensor_tensor(out=tmpje[:], in0=iotaf_sb[:],
                                                   scalar=-9.0, in1=eq[:],
                                                   op0=Alu.add, op1=Alu.mult)
                    nc.vector.tensor_scalar_add(tmpje[:], tmpje[:], 9.0)
                    nc.vector.tensor_reduce(out=idx2[:], in_=tmpje[:],
                                            axis=mybir.AxisListType.X, op=Alu.min)

                    # rw1 = sigmoid((m1-m2)*rms), rw2 = 1-rw1
                    nc.vector.tensor_sub(tmp[:], m1[:], m2[:])
                    nc.vector.tensor_mul(tmp[:], tmp[:], rms_all[:])
                    nc.scalar.activation(out=rw1[:], in_=tmp[:], func=Act.Sigmoid)
                    nc.vector.tensor_scalar(rw2[:], rw1[:], -1.0, 1.0,
                                            op0=Alu.mult, op1=Alu.add)

                    se1 = r2.tile([P, J], f32)
                    se2 = r2.tile([P, J], f32)
                    sel = r2.tile([P, J], f32)
                    nc.vector.tensor_tensor(out=se1[:], in0=idx1[:], in1=eidf_sb[:],
                                            op=Alu.is_equal)
                    nc.vector.tensor_tensor(out=se2[:], in0=idx2[:], in1=eidf_sb[:],
                                            op=Alu.is_equal)
                    nc.vector.tensor_add(sel[:], se1[:], se2[:])
                    nc.vector.tensor_mul(se1[:], se1[:], rw1[:])
                    nc.vector.tensor_mul(se2[:], se2[:], rw2[:])
                    nc.vector.tensor_add(wv_pm[:], se1[:], se2[:])

                    # ------------- R3: capacity dispatch ------------------------
                    selT_ps = r2ps.tile([J, P], f32)
                    nc.tensor.transpose(selT_ps[:], sel[:], ident[:])
                    selT = r2.tile([J, P], f32)
                    nc.vector.tensor_copy(out=selT[:], in_=selT_ps[:])
                    zerosT = r2.tile([J, P], f32)
                    nc.vector.memset(zerosT[:], 0.0)
                    rowsum = r2.tile([J, 1], f32)
                    nc.vector.tensor_reduce(out=rowsum[:], in_=selT[:],
                                            axis=mybir.AxisListType.X, op=Alu.add)
                    offs_ps = r2ps.tile([J, 1], f32)
                    nc.tensor.matmul(out=offs_ps[:], lhsT=utri_sb[:], rhs=rowsum[:],
                                     start=True, stop=True)
                    scanT = r2.tile([J, P], f32)
                    nc.vector.tensor_tensor_scan(out=scanT[:], data0=selT[:],
                                                 data1=zerosT[:],
                                                 initial=offs_ps[:],
                                                 op0=Alu.add, op1=Alu.add)
                    nc.vector.tensor_scalar_add(scanT[:], scanT[:], -1.0)
                    slot_ps = r2ps.tile([P, J], f32)
                    nc.tensor.transpose(slot_ps[:], scanT[:], ident[:J, :J])
                    slot_pm = r2.tile([P, J], f32)
                    nc.vector.tensor_copy(out=slot_pm[:], in_=slot_ps[:])

                    # wv *= (slot < CAP)
                    gate = r2.tile([P, J], f32)
                    nc.vector.tensor_scalar(gate[:], slot_pm[:], float(CAP), None,
                                            op0=Alu.is_lt)
                    nc.vector.tensor_mul(wv_pm[:], wv_pm[:], gate[:])
                    # gather slot: clamp to [0, CAP-1]
                    sg_f = r2.tile([P, J], f32)
                    nc.vector.tensor_scalar(sg_f[:], slot_pm[:], 0.0, float(CAP - 1),
                                            op0=Alu.max, op1=Alu.min)
                    nc.vector.tensor_copy(out=slotg_i32[:], in_=sg_f[:])
                    # dispatch build: stream-compact (sel & slot<CAP ? tokid : -1)
                    # in token order via gpsimd sparse_gather. The capacity cap
                    # keeps the found count <= CAP so the ucode cannot overrun
                    # its [16, CAP/16] output.
                    selcap = r2.tile([P, J], f32)
                    nc.vector.tensor_mul(selcap[:], sel[:], gate[:])
                    val_pm = r2.tile([P, J], f32)
                    nc.vector.scalar_tensor_tensor(out=val_pm[:], in0=tokidf_sb[:],
                                                   scalar=1.0, in1=selcap[:],
                                                   op0=Alu.add, op1=Alu.mult)
                    nc.vector.tensor_scalar_add(val_pm[:], val_pm[:], -1.0)
                    nc.sync.dma_start(
                        out=vlin_dram[:].rearrange("(j p) -> p j", p=P),
                        in_=val_pm[:])
                    v16 = r2.tile([16, T // 16], f32)
                    nc.sync.dma_start(out=v16[:],
                                      in_=vlin_dram[:].rearrange("(f p) -> p f", p=16))
                    d16 = r2.tile([16, CAP // 16], f32)
                    nfound = r2.tile([1, 1], dt.uint32)
                    nc.gpsimd.sparse_gather(out=d16[:], in_=v16[:],
                                            num_found=nfound[:])
                    nc.sync.dma_start(
                        out=dlin_dram[:].rearrange("(f p) -> p f", p=16),
                        in_=d16[:])
                    dispf = r2.tile([P, CT], f32)
                    nc.sync.dma_start(
                        out=dispf[:],
                        in_=dlin_dram[:].rearrange("(ct p) -> p ct", p=P))
                    nfoundf = r2.tile([1, 1], f32)
                    nc.vector.tensor_copy(out=nfoundf[:], in_=nfound[:])
                    cntb = r2.tile([P, 1], f32)
                    nc.gpsimd.partition_broadcast(cntb[:], nfoundf[:])
                    cgate = r2.tile([P, CT], dt.uint8)
                    nc.vector.tensor_scalar(cgate[:], slotiota_sb[:], cntb[:],
                                            None, op0=Alu.is_lt)
                    zct = r2.tile([P, CT], f32)
                    nc.vector.memset(zct[:], 0.0)
                    dsafe = r2.tile([P, CT], f32)
                    nc.vector.select(out=dsafe[:], mask=cgate[:], on_true=dispf[:],
                                     on_false=zct[:])
                    nc.vector.tensor_copy(out=disp_sb[:], in_=dsafe[:])

                # remaining shared-gu supertiles run during R2/R3 + P2 prep
                for st in range(GI + GPOST, NST):
                    gu_supertile(st, load_xT(xht_in, st, "xTg"))

            # ---------------- P2: routed expert FFN -------------------------
            with tc.tile_pool(name="p2", bufs=3) as p2, \
                 tc.tile_pool(name="p2big", bufs=1) as p2big:
                nc.sync.dma_start(
                    out=sdw[:],
                    in_=sd_in[:].rearrange("(it ip) h -> ip it h", ip=P))
                xgT = p2big.tile([P, HC, CAP], f16)
                hT = p2big.tile([P, IRT, CAP], f16)
                with tc.tile_pool(name="p2psA", bufs=1, space="PSUM") as psa:
                    for ct in range(CT):
                        xg_sb = p2.tile([P, H], f16, tag="xg_sb")
                        nc.gpsimd.indirect_dma_start(
                            out=xg_sb[:], out_offset=None,
                            in_=xh_in[:, :],
                            in_offset=bass.IndirectOffsetOnAxis(
                                ap=disp_sb[:, ct:ct + 1], axis=0))
                        nc.sync.dma_start_transpose(
                            out=xgT[:, :, ct * P:(ct + 1) * P], in_=xg_sb[:])

                    cchunks = []
                    c0 = 0
                    while c0 < CAP:
                        cw = min(512, CAP - c0)
                        cchunks.append((c0, cw))
                        c0 += cw
                    for irt in range(IRT):
                        rgw = p2.tile([P, HC, P], f16, tag="rgw")
                        nc.sync.dma_start(
                            out=rgw[:],
                            in_=rg_in[:, irt * P:(irt + 1) * P].rearrange(
                                "(hc hp) i -> hp hc i", hp=P))
                        ruw = p2.tile([P, HC, P], f16, tag="ruw")
                        nc.sync.dma_start(
                            out=ruw[:],
                            in_=ru_in[:, irt * P:(irt + 1) * P].rearrange(
                                "(hc hp) i -> hp hc i", hp=P))
                        for (c0, cw) in cchunks:
                            ps_g = psa.tile([P, 512], f32, tag="ps_g", bufs=2)
                            ps_u = psa.tile([P, 512], f32, tag="ps_u", bufs=2)
                            for hc in range(HC):
                                nc.tensor.matmul(
                                    out=ps_g[:, :cw], lhsT=rgw[:, hc, :],
                                    rhs=xgT[:, hc, c0:c0 + cw],
                                    start=(hc == 0), stop=(hc == HC - 1))
                                nc.tensor.matmul(
                                    out=ps_u[:, :cw], lhsT=ruw[:, hc, :],
                                    rhs=xgT[:, hc, c0:c0 + cw],
                                    start=(hc == 0), stop=(hc == HC - 1))
                            sil = p2.tile([P, 512], f32, tag="sil")
                            nc.scalar.activation(out=sil[:, :cw],
                                                 in_=ps_g[:, :cw],
                                                 func=Act.Sigmoid)
                            nc.vector.tensor_tensor(
                                out=sil[:, :cw], in0=sil[:, :cw],
                                in1=ps_g[:, :cw], op=Alu.mult)
                            nc.vector.tensor_tensor(
                                out=hT[:, irt, c0:c0 + cw], in0=sil[:, :cw],
                                in1=ps_u[:, :cw], op=Alu.mult)

                # down: eout[c, h] = sum_ir h[c, ir] * rd[ir, h]
                with tc.tile_pool(name="p2psD", bufs=1, space="PSUM") as psd:
                    ct0 = 0
                    while ct0 < CT:
                        blk = min(4, CT - ct0)
                        ps_d = [[psd.tile([P, 512], f32, tag=f"d{i}{hn}",
                                          name=f"d{i}{hn}")
                                 for hn in range(NHT)] for i in range(blk)]
                        for ic in range(IRT):
                            rdw = p2.tile([P, H], f16, tag="rdw")
                            nc.sync.dma_start(
                                out=rdw[:], in_=rd_in[ic * P:(ic + 1) * P, :])
                            for i in range(blk):
                                for hn in range(NHT):
                                    nc.tensor.matmul(
                                        out=ps_d[i][hn][:],
                                        lhsT=hT[:, ic,
                                                (ct0 + i) * P:(ct0 + i + 1) * P],
                                        rhs=rdw[:, hn * 512:(hn + 1) * 512],
                                        start=(ic == 0), stop=(ic == IRT - 1))
                        for i in range(blk):
                            eo = p2.tile([P, H], f16, tag="eo")
                            for hn in range(NHT):
                                nc.vector.tensor_copy(
                                    out=eo[:, hn * 512:(hn + 1) * 512],
                                    in_=ps_d[i][hn][:])
                            nc.sync.dma_start(
                                out=eout_dram[(ct0 + i) * P:(ct0 + i + 1) * P, :],
                                in_=eo[:])
                        ct0 += blk

            # ------- P3: shared down-proj + combine + ReduceScatter ---------
            with tc.tile_pool(name="p3", bufs=2) as p3, \
                 tc.tile_pool(name="p3ps", bufs=1, space="PSUM") as p3ps:
                GPF = 2  # supertiles of gather prefetch ahead of RS issue

                def gather_st(st):
                    tiles = []
                    for ts in range(ST):
                        j = st * ST + ts
                        gath = p3.tile([P, H], f16, tag="gath", bufs=4 * GPF + 4)
                        nc.gpsimd.indirect_dma_start(
                            out=gath[:], out_offset=None,
                            in_=eout_dram[:, :],
                            in_offset=bass.IndirectOffsetOnAxis(
                                ap=slotg_i32[:, j:j + 1], axis=0))
                        tiles.append(gath)
                    return tiles

                hs_tiles = {}

                def load_hs(st):
                    hsT = p3.tile([P, ISHT, STT], f16, tag="hsS", bufs=3)
                    nc.sync.dma_start(out=hsT[:], in_=hs_dram[st])
                    hs_tiles[st] = hsT

                gath_tiles = {}
                for st in range(GPF):
                    gath_tiles[st] = gather_st(st)
                load_hs(0)
                load_hs(1)

                for st in range(NST):
                    hsT = hs_tiles.pop(st)
                    if st + 2 < NST:
                        load_hs(st + 2)
                    for ts in range(ST):
                        j = st * ST + ts
                        gath = gath_tiles[st][ts]
                        outt = p3.tile([P, H], f16, tag="outt")
                        for hn in range(NHT):
                            ps_d3 = p3ps.tile([P, 512], f32, tag="ps_d3",
                                              bufs=2)
                            for it in range(ISHT):
                                nc.tensor.matmul(
                                    out=ps_d3[:],
                                    lhsT=hsT[:, it, ts * P:(ts + 1) * P],
                                    rhs=sdw[:, it, hn * 512:(hn + 1) * 512],
                                    start=(it == 0), stop=(it == ISHT - 1))
                            nc.vector.scalar_tensor_tensor(
                                out=outt[:, hn * 512:(hn + 1) * 512],
                                in0=gath[:, hn * 512:(hn + 1) * 512],
                                scalar=wv_pm[:, j:j + 1],
                                in1=ps_d3[:],
                                op0=Alu.mult, op1=Alu.add)
                        k = j // JCH
                        r = j % JCH
                        nc.sync.dma_start(
                            out=chunk_dram[k][r * P:(r + 1) * P, :],
                            in_=outt[:])
                    del gath_tiles[st]
                    if st + GPF < NST:
                        gath_tiles[st + GPF] = gather_st(st + GPF)
                    # one supertile == one RS chunk (NST == NCH)
                    nc.gpsimd.collective_compute(
                        "ReduceScatter", Alu.add,
                        replica_groups=replica_groups,
                        ins=[chunk_dram[st][:]],
                        outs=[rsout_dram[st][:]])
                    # fp16 -> f32 cast on the way out (SWDGE), one chunk late
                    # so the cast's RS-wait cannot delay the next gathers
                    if st > 0:
                        nc.gpsimd.dma_start(out=o_out[st - 1],
                                            in_=rsout_dram[st - 1][:])
                nc.gpsimd.dma_start(out=o_out[NST - 1],
                                    in_=rsout_dram[NST - 1][:])

    nc.finalize()
    return nc


_NC_CACHE = {}


def _get_nc(key="full"):
    if key not in _NC_CACHE:
        _NC_CACHE[key] = build(FULL_CFG)
    return _NC_CACHE[key]


def make_in_maps(inputs, cfg=FULL_CFG):
    T, H, E, IR, IS, CAP, NCORES = (
        cfg[k] for k in ("T", "H", "E", "IR", "IS", "CAP", "NC"))
    J = T // P
    ISH = IS // NCORES
    f16 = np.float16
    x = np.ascontiguousarray(np.asarray(inputs["x"], np.float32).reshape(T, H))
    xh = x.astype(f16)
    xl = (x - xh.astype(np.float32)).astype(f16)
    xht = np.ascontiguousarray(xh.T)
    xlt = np.ascontiguousarray(xl.T)
    rw = np.asarray(inputs["router_w"], np.float32)
    rwt = np.ascontiguousarray(rw.T)
    rwh = rwt.astype(f16)
    rwl = (rwt - rwh.astype(np.float32)).astype(f16)
    rw16 = np.ascontiguousarray(np.concatenate([rwh, rwl], axis=1))
    rg = np.asarray(inputs["rg"], np.float32)
    ru = np.asarray(inputs["ru"], np.float32)
    rd = np.asarray(inputs["rd"], np.float32)
    sg = np.asarray(inputs["sg"], np.float32)
    su = np.asarray(inputs["su"], np.float32)
    sd = np.asarray(inputs["sd"], np.float32)

    tokidf = np.ascontiguousarray(
        np.arange(T, dtype=np.float32).reshape(J, P).T)      # [p, j] = 128j+p
    slotiota = np.ascontiguousarray(
        np.arange(CAP, dtype=np.float32).reshape(CAP // P, P).T)  # [p, ct]
    utri = np.triu(np.ones((J, J), np.float32), k=1)
    iotaf = np.tile(np.arange(E, dtype=np.float32), (P, J))  # [P, J*E]

    in_maps = []
    for i in range(NCORES):
        in_maps.append(dict(
            xh=xh, xht=xht, xlt=xlt, rw16=rw16,
            rg=np.ascontiguousarray(rg[i]).astype(f16),
            ru=np.ascontiguousarray(ru[i]).astype(f16),
            rd=np.ascontiguousarray(rd[i]).astype(f16),
            sg=np.ascontiguousarray(sg[:, i * ISH:(i + 1) * ISH]).astype(f16),
            su=np.ascontiguousarray(su[:, i * ISH:(i + 1) * ISH]).astype(f16),
            sd=np.ascontiguousarray(sd[i * ISH:(i + 1) * ISH, :]).astype(f16),
            eidf=np.full((P, J), float(i), np.float32),
            iotaf=iotaf, tokidf=tokidf, slotiota=slotiota, utri=utri,
        ))
    return in_maps


def assemble_output(results, cfg=FULL_CFG):
    T, H, NCORES = cfg["T"], cfg["H"], cfg["NC"]
    NCH = cfg.get("NCH", 16)
    rows_per = T // NCH // NCORES
    out = np.empty((T, H), np.float32)
    for i in range(NCORES):
        o = results[i]["o"]
        for k in range(NCH):
            base = (T // NCH) * k + rows_per * i
            out[base:base + rows_per] = o[k]
    return out


def kernel(**inputs):
    nc = _get_nc()
    in_maps = make_in_maps(inputs)
    core_ids = list(range(FULL_CFG["NC"]))
    last_err = None
    for _attempt in range(2):
        try:
            res = run_bass_kernel_spmd(nc, in_maps, core_ids, trace=False)
            break
        except Exception as e:  # transient device wedges: retry once
            last_err = e
    else:
        raise last_err
    out = assemble_output(res.results)
    B, S, H = 4, 2048, 1024
    return out.reshape(B, S, H)


# revision 15
# speedup vs baseline: 1.1211x; 1.1211x over previous
"""DeepSeek-MoE layer on 8 Trainium2 NeuronCores (v3: merged phases).

Sharding: expert-parallel routed experts (1 expert/core, full x replicated so
no token all-to-all is needed), tensor-parallel shared expert (I_S split 8
ways), single ReduceScatter (16 pipelined fp16 chunks) combines routed +
shared partial sums; each core holds 1/16*1/8 row chunks of the output.

v3 changes vs v2 (1.80ms -> target ~1.35ms):
  - Phase A merges the router pass with the shared-expert gate/up compute:
    the DMA-bound router window (xht/xlt/xh streams) is filled with 3
    interleaved shared-gu supertiles, the remaining 13 run while R2/R3
    (top-2 + capacity dispatch) executes on Vector/GpSimd, so the PE never
    idles during routing.  hsT supertiles spill to DRAM (cheap DMA) and are
    streamed back in P3 where the shared down-proj fuses with the
    routed-combine + ReduceScatter, exactly as v2.
  - Router fp16-split passes packed: lhsT [rwh|rwl] computes wh*xh and
    wl*xh in one matmul group half, wh*xl accumulated on rows 0:8;
    transpose [16] columns then one vector add folds the halves
    (24 -> 16 matmuls per supertile).
  - P3 gathers prefetch 2 supertiles ahead of each ReduceScatter issue so
    the first RS's inter-core skew wait cannot starve the combine; output
    cast DMAs are issued one chunk late for the same reason.
"""

import numpy as np
import ml_dtypes

import concourse.bass as bass
import concourse.mybir as mybir
import concourse.tile as tile
from concourse import bacc
from concourse.masks import make_identity
from concourse.bass_utils import run_bass_kernel_spmd

dt = mybir.dt
Alu = mybir.AluOpType
Act = mybir.ActivationFunctionType

P = 128

FULL_CFG = dict(T=8192, H=1024, E=8, IR=4096, IS=8192, CAP=1280, NC=8, NCH=16)


def build(cfg):
    T, H, E, IR, IS, CAP, NCORES = (
        cfg[k] for k in ("T", "H", "E", "IR", "IS", "CAP", "NC")
    )
    J = T // P            # 128-token tiles
    HC = H // P           # h chunks of 128
    ISH = IS // NCORES    # shared-expert intermediate shard
    CT = CAP // P         # capacity tiles of 128 slots
    NCH = cfg.get("NCH", 16)  # reduce-scatter chunks
    JCH = J // NCH        # token tiles per RS chunk
    ST = 4                # token tiles per shared-expert supertile
    STT = ST * P
    NST = J // ST
    assert J % NCH == 0 and J % ST == 0 and CAP % P == 0
    IRT = IR // P
    ISHT = ISH // P
    NHT = H // 512
    GI = 3                # shared-gu supertiles interleaved into router window
    GPRE = 2              # supertiles emitted between router end and R2/R3
    GPOST = 2             # supertiles emitted after R2/R3 (cover P2 prep)
    NSPILL = GI + GPRE + GPOST  # supertiles whose hsT spills to DRAM

    nc = bacc.Bacc(None)

    f32, f16, i32 = dt.float32, dt.float16, dt.int32

    xh_in = nc.declare_dram_parameter("xh", [T, H], f16, isOutput=False)
    xht_in = nc.declare_dram_parameter("xht", [H, T], f16, isOutput=False)
    xlt_in = nc.declare_dram_parameter("xlt", [H, T], f16, isOutput=False)
    rw16_in = nc.declare_dram_parameter("rw16", [H, 2 * E], f16, isOutput=False)
    rg_in = nc.declare_dram_parameter("rg", [H, IR], f16, isOutput=False)
    ru_in = nc.declare_dram_parameter("ru", [H, IR], f16, isOutput=False)
    rd_in = nc.declare_dram_parameter("rd", [IR, H], f16, isOutput=False)
    sg_in = nc.declare_dram_parameter("sg", [H, ISH], f16, isOutput=False)
    su_in = nc.declare_dram_parameter("su", [H, ISH], f16, isOutput=False)
    sd_in = nc.declare_dram_parameter("sd", [ISH, H], f16, isOutput=False)
    eidf_in = nc.declare_dram_parameter("eidf", [P, J], f32, isOutput=False)
    iotaf_in = nc.declare_dram_parameter("iotaf", [P, J * E], f32, isOutput=False)
    tokidf_in = nc.declare_dram_parameter("tokidf", [P, J], f32, isOutput=False)
    slotiota_in = nc.declare_dram_parameter("slotiota", [P, CAP // P], f32, isOutput=False)
    utri_in = nc.declare_dram_parameter("utri", [J, J], f32, isOutput=False)
    o_out = nc.declare_dram_parameter("o", [NCH, T // NCH // NCORES, H], f32,
                                      isOutput=True)

    replica_groups = [list(range(NCORES))]

    with tile.TileContext(nc) as tc:
        with tc.tile_pool(name="dram", bufs=1, space="DRAM") as drp, \
             tc.tile_pool(name="pers", bufs=1) as pers:
            vlin_dram = drp.tile([T], f32)
            dlin_dram = drp.tile([CAP], f32)
            eout_dram = drp.tile([CAP, H], f16)
            hs_dram = drp.tile([NST, P, ISHT * STT], f16)
            chunk_dram = [drp.tile([T // NCH, H], f16, name=f"partial{k}")
                          for k in range(NCH)]
            rsout_dram = [drp.tile([T // NCH // NCORES, H], f16,
                                   name=f"rsout{k}") for k in range(NCH)]

            ident = pers.tile([P, P], f32)
            make_identity(nc, ident[:])
            rw16_sb = pers.tile([P, HC, 2 * E], f16)
            nc.sync.dma_start(out=rw16_sb[:],
                              in_=rw16_in[:].rearrange("(hc hp) e -> hp hc e", hp=P))
            eidf_sb = pers.tile([P, J], f32)
            nc.sync.dma_start(out=eidf_sb[:], in_=eidf_in[:])
            iotaf_sb = pers.tile([P, J, E], f32)
            nc.sync.dma_start(out=iotaf_sb[:],
                              in_=iotaf_in[:].rearrange("p (j e) -> p j e", e=E))
            tokidf_sb = pers.tile([P, J], f32)
            nc.sync.dma_start(out=tokidf_sb[:], in_=tokidf_in[:])
            slotiota_sb = pers.tile([P, CT], f32)
            nc.sync.dma_start(out=slotiota_sb[:], in_=slotiota_in[:])
            utri_sb = pers.tile([J, J], f32)
            nc.sync.dma_start(out=utri_sb[:], in_=utri_in[:])

            eps_sb = pers.tile([P, 1], f32)
            nc.vector.memset(eps_sb[:], float(np.finfo(np.float32).eps))

            z_all = pers.tile([P, J, E], f32)
            ss_all = pers.tile([P, J], f32)
            rms_all = pers.tile([P, J], f32)
            wv_pm = pers.tile([P, J], f32)       # combine weight per token
            slotg_i32 = pers.tile([P, J], i32)   # clamped slot for gather
            disp_sb = pers.tile([P, CT], i32)    # dispatch token ids

            def shared_gu(xTs, hsT, gw, uw, spool, pspool):
                """gate/up + silu for one shared-expert supertile."""
                for it in range(ISHT):
                    ps_g3 = pspool.tile([P, STT], f32, tag="ps_g3", bufs=2)
                    ps_u3 = pspool.tile([P, STT], f32, tag="ps_u3", bufs=2)
                    for hc in range(HC):
                        nc.tensor.matmul(
                            out=ps_g3[:], lhsT=gw[:, hc, it * P:(it + 1) * P],
                            rhs=xTs[:, hc, :],
                            start=(hc == 0), stop=(hc == HC - 1))
                        nc.tensor.matmul(
                            out=ps_u3[:], lhsT=uw[:, hc, it * P:(it + 1) * P],
                            rhs=xTs[:, hc, :],
                            start=(hc == 0), stop=(hc == HC - 1))
                    sil3 = spool.tile([P, STT], f32, tag="sil3", bufs=2)
                    nc.scalar.activation(out=sil3[:], in_=ps_g3[:],
                                         func=Act.Silu)
                    nc.vector.tensor_tensor(out=hsT[:, it, :], in0=sil3[:],
                                            in1=ps_u3[:], op=Alu.mult)

            # ============ Phase A: router + shared gate/up, merged ===========
            with tc.tile_pool(name="phA", bufs=1) as phA, \
                 tc.tile_pool(name="gups", bufs=1, space="PSUM") as gups:
                sgw = phA.tile([P, HC, ISH], f16)
                nc.sync.dma_start(
                    out=sgw[:], in_=sg_in[:].rearrange("(hc hp) i -> hp hc i", hp=P))
                suw = phA.tile([P, HC, ISH], f16)
                nc.sync.dma_start(
                    out=suw[:], in_=su_in[:].rearrange("(hc hp) i -> hp hc i", hp=P))

                def load_xT(src, st, tag, bufs=3):
                    t0 = st * STT
                    xT = phA.tile([P, HC, STT], f16, tag=tag, bufs=bufs)
                    nc.sync.dma_start(
                        out=xT[:],
                        in_=src[:, t0:t0 + STT].rearrange(
                            "(hc hp) t -> hp hc t", hp=P))
                    return xT

                def gu_supertile(st, xTs):
                    hsT = phA.tile([P, ISHT, STT], f16, tag="hsT", bufs=2)
                    shared_gu(xTs, hsT, sgw, suw, phA, gups)
                    nc.sync.dma_start(out=hs_dram[st], in_=hsT[:])

                with tc.tile_pool(name="rtps", bufs=1, space="PSUM") as rtps:
                    for st in range(NST):
                        xTh = load_xT(xht_in, st, "xTh")
                        xTl = load_xT(xlt_in, st, "xTl")
                        # router: z = wh*xh + wl*xh (packed rows) then wh*xl
                        zps = rtps.tile([2 * E, STT], f32, tag="zps", bufs=2)
                        k, n_mm = 0, 2 * HC
                        for hc in range(HC):
                            nc.tensor.matmul(out=zps[:], lhsT=rw16_sb[:, hc, :],
                                             rhs=xTh[:, hc, :],
                                             start=(k == 0), stop=False)
                            k += 1
                        for hc in range(HC):
                            nc.tensor.matmul(out=zps[:E, :],
                                             lhsT=rw16_sb[:, hc, :E],
                                             rhs=xTl[:, hc, :],
                                             start=False, stop=(k == n_mm - 1),
                                             skip_group_check=True)
                            k += 1
                        ztmp = phA.tile([2 * E, STT], f32, tag="ztmp", bufs=2)
                        nc.vector.tensor_copy(out=ztmp[:], in_=zps[:])
                        for q in range(ST):
                            ztr = rtps.tile([P, 2 * E], f32, tag="ztr", bufs=2)
                            nc.tensor.transpose(ztr[:], ztmp[:, q * P:(q + 1) * P],
                                                ident[:2 * E, :2 * E])
                            ztr_sb = phA.tile([P, 2 * E], f32, tag="ztr_sb",
                                              bufs=2)
                            nc.vector.tensor_copy(out=ztr_sb[:], in_=ztr[:])
                            nc.vector.tensor_tensor(
                                out=z_all[:, st * ST + q, :],
                                in0=ztr_sb[:, :E], in1=ztr_sb[:, E:], op=Alu.add)
                        # sum(x^2) stream for the RMS scale (scalar engine)
                        for q in range(ST):
                            j = st * ST + q
                            xr = phA.tile([P, H], f16, tag="xr", bufs=6)
                            nc.sync.dma_start(out=xr[:],
                                              in_=xh_in[j * P:(j + 1) * P, :])
                            sq = phA.tile([P, H], f32, tag="sq", bufs=2)
                            nc.scalar.activation(out=sq[:], in_=xr[:],
                                                 func=Act.Square,
                                                 accum_out=ss_all[:, j:j + 1])
                        if st < GI:
                            gu_supertile(st, xTh)
                    # bridge supertiles: cover R2's latency (xht reload)
                    for st in range(GI, GI + GPRE):
                        gu_supertile(st, load_xT(xht_in, st, "xTg"))

                # ---------------- R2: top-2 + weights -----------------------
                with tc.tile_pool(name="r2", bufs=1) as r2, \
                     tc.tile_pool(name="r2ps", bufs=1, space="PSUM") as r2ps:
                    srt = r2.tile([P, J], f32)
                    nc.scalar.activation(out=srt[:], in_=ss_all[:], func=Act.Sqrt,
                                         scale=1.0 / H, bias=eps_sb[:])
                    nc.vector.reciprocal(out=rms_all[:], in_=srt[:])

                    m1 = r2.tile([P, J], f32)
                    m2 = r2.tile([P, J], f32)
                    idx1 = r2.tile([P, J], f32)
                    idx2 = r2.tile([P, J], f32)
                    eq = r2.tile([P, J, E], f32)
                    tmpje = r2.tile([P, J, E], f32)
                    tmp = r2.tile([P, J], f32)
                    rw1 = r2.tile([P, J], f32)
                    rw2 = r2.tile([P, J], f32)

                    nc.vector.tensor_reduce(out=m1[:], in_=z_all[:],
                                            axis=mybir.AxisListType.X, op=Alu.max)
                    m1b = m1[:].rearrange("p j -> p j ()").to_broadcast([P, J, E])
                    nc.vector.tensor_tensor(out=eq[:], in0=z_all[:], in1=m1b,
                                            op=Alu.is_ge)
                    # idx1 = min over e of (eq ? iota : 9)
                    nc.vector.scalar_tensor_tensor(out=tmpje[:], in0=iotaf_sb[:],
                                                   scalar=-9.0, in1=eq[:],
                                                   op0=Alu.add, op1=Alu.mult)
                    nc.vector.tensor_scalar_add(tmpje[:], tmpje[:], 9.0)
                    nc.vector.tensor_reduce(out=idx1[:], in_=tmpje[:],
                                            axis=mybir.AxisListType.X, op=Alu.min)
                    # mask out the top-1 positions, then find second max
                    nc.vector.scalar_tensor_tensor(out=eq[:], in0=eq[:],
                                                   scalar=-1e30, in1=z_all[:],
                                                   op0=Alu.mult, op1=Alu.add)
                    nc.vector.tensor_reduce(out=m2[:], in_=eq[:],
                                            axis=mybir.AxisListType.X, op=Alu.max)
                    m2b = m2[:].rearrange("p j -> p j ()").to_broadcast([P, J, E])
                    nc.vector.tensor_tensor(out=eq[:], in0=eq[:], in1=m2b,
                                            op=Alu.is_ge)
                    nc.vector.scalar_tensor_tensor(out=tmpje[:], in0=iotaf_sb[:],
                                                   scalar=-9.0, in1=eq[:],
                                                   op0=Alu.add, op1=Alu.mult)
                    nc.vector.tensor_scalar_add(tmpje[:], tmpje[:], 9.0)
                    nc.vector.tensor_reduce(out=idx2[:], in_=tmpje[:],
                                            axis=mybir.AxisListType.X, op=Alu.min)

                    # rw1 = sigmoid((m1-m2)*rms), rw2 = 1-rw1
                    nc.vector.tensor_sub(tmp[:], m1[:], m2[:])
                    nc.vector.tensor_mul(tmp[:], tmp[:], rms_all[:])
                    nc.scalar.activation(out=rw1[:], in_=tmp[:], func=Act.Sigmoid)
                    nc.vector.tensor_scalar(rw2[:], rw1[:], -1.0, 1.0,
                                            op0=Alu.mult, op1=Alu.add)

                    se1 = r2.tile([P, J], f32)
                    se2 = r2.tile([P, J], f32)
                    sel = r2.tile([P, J], f32)
                    nc.vector.tensor_tensor(out=se1[:], in0=idx1[:], in1=eidf_sb[:],
                                            op=Alu.is_equal)
                    nc.vector.tensor_tensor(out=se2[:], in0=idx2[:], in1=eidf_sb[:],
                                            op=Alu.is_equal)
                    nc.vector.tensor_add(sel[:], se1[:], se2[:])
                    nc.vector.tensor_mul(se1[:], se1[:], rw1[:])
                    nc.vector.tensor_mul(se2[:], se2[:], rw2[:])
                    nc.vector.tensor_add(wv_pm[:], se1[:], se2[:])

                    # ------------- R3: capacity dispatch ------------------------
                    selT_ps = r2ps.tile([J, P], f32)
                    nc.tensor.transpose(selT_ps[:], sel[:], ident[:])
                    selT = r2.tile([J, P], f32)
                    nc.vector.tensor_copy(out=selT[:], in_=selT_ps[:])
                    zerosT = r2.tile([J, P], f32)
                    nc.vector.memset(zerosT[:], 0.0)
                    rowsum = r2.tile([J, 1], f32)
                    nc.vector.tensor_reduce(out=rowsum[:], in_=selT[:],
                                            axis=mybir.AxisListType.X, op=Alu.add)
                    offs_ps = r2ps.tile([J, 1], f32)
                    nc.tensor.matmul(out=offs_ps[:], lhsT=utri_sb[:], rhs=rowsum[:],
                                     start=True, stop=True)
                    scanT = r2.tile([J, P], f32)
                    nc.vector.tensor_tensor_scan(out=scanT[:], data0=selT[:],
                                                 data1=zerosT[:],
                                                 initial=offs_ps[:],
                                                 op0=Alu.add, op1=Alu.add)
                    nc.vector.tensor_scalar_add(scanT[:], scanT[:], -1.0)
                    slot_ps = r2ps.tile([P, J], f32)
                    nc.tensor.transpose(slot_ps[:], scanT[:], ident[:J, :J])
                    slot_pm = r2.tile([P, J], f32)
                    nc.vector.tensor_copy(out=slot_pm[:], in_=slot_ps[:])

                    # wv *= (slot < CAP)
                    gate = r2.tile([P, J], f32)
                    nc.vector.tensor_scalar(gate[:], slot_pm[:], float(CAP), None,
                                            op0=Alu.is_lt)
                    nc.vector.tensor_mul(wv_pm[:], wv_pm[:], gate[:])
                    # gather slot: clamp to [0, CAP-1]
                    sg_f = r2.tile([P, J], f32)
                    nc.vector.tensor_scalar(sg_f[:], slot_pm[:], 0.0, float(CAP - 1),
                                            op0=Alu.max, op1=Alu.min)
                    nc.vector.tensor_copy(out=slotg_i32[:], in_=sg_f[:])
                    # dispatch build: stream-compact (sel & slot<CAP ? tokid : -1)
                    # in token order via gpsimd sparse_gather. The capacity cap
                    # keeps the found count <= CAP so the ucode cannot overrun
                    # its [16, CAP/16] output.
                    selcap = r2.tile([P, J], f32)
                    nc.vector.tensor_mul(selcap[:], sel[:], gate[:])
                    val_pm = r2.tile([P, J], f32)
                    nc.vector.scalar_tensor_tensor(out=val_pm[:], in0=tokidf_sb[:],
                                                   scalar=1.0, in1=selcap[:],
                                                   op0=Alu.add, op1=Alu.mult)
                    nc.vector.tensor_scalar_add(val_pm[:], val_pm[:], -1.0)
                    # round trips ride the scalar queue so they cannot clog
                    # the sync queue (phase-A loads / P2 weights + transposes)
                    nc.scalar.dma_start(
                        out=vlin_dram[:].rearrange("(j p) -> p j", p=P),
                        in_=val_pm[:])
                    v16 = r2.tile([16, T // 16], f32)
                    nc.scalar.dma_start(out=v16[:],
                                        in_=vlin_dram[:].rearrange("(f p) -> p f", p=16))
                    d16 = r2.tile([16, CAP // 16], f32)
                    nfound = r2.tile([1, 1], dt.uint32)
                    nc.gpsimd.sparse_gather(out=d16[:], in_=v16[:],
                                            num_found=nfound[:])
                    nc.scalar.dma_start(
                        out=dlin_dram[:].rearrange("(f p) -> p f", p=16),
                        in_=d16[:])
                    dispf = r2.tile([P, CT], f32)
                    nc.scalar.dma_start(
                        out=dispf[:],
                        in_=dlin_dram[:].rearrange("(ct p) -> p ct", p=P))
                    nfoundf = r2.tile([1, 1], f32)
                    nc.vector.tensor_copy(out=nfoundf[:], in_=nfound[:])
                    cntb = r2.tile([P, 1], f32)
                    nc.gpsimd.partition_broadcast(cntb[:], nfoundf[:])
                    cgate = r2.tile([P, CT], dt.uint8)
                    nc.vector.tensor_scalar(cgate[:], slotiota_sb[:], cntb[:],
                                            None, op0=Alu.is_lt)
                    zct = r2.tile([P, CT], f32)
                    nc.vector.memset(zct[:], 0.0)
                    dsafe = r2.tile([P, CT], f32)
                    nc.vector.select(out=dsafe[:], mask=cgate[:], on_true=dispf[:],
                                     on_false=zct[:])
                    nc.vector.tensor_copy(out=disp_sb[:], in_=dsafe[:])

                # bridge supertiles covering R3's tail + P2's gather/transpose
                for st in range(GI + GPRE, NSPILL):
                    gu_supertile(st, load_xT(xht_in, st, "xTg"))

            # ---------------- P2: routed expert FFN -------------------------
            with tc.tile_pool(name="p2", bufs=3) as p2, \
                 tc.tile_pool(name="p2big", bufs=1) as p2big:
                xgT = p2big.tile([P, HC, CAP], f16)
                hT = p2big.tile([P, IRT, CAP], f16)
                with tc.tile_pool(name="p2psA", bufs=1, space="PSUM") as psa:
                    for ct in range(CT):
                        xg_sb = p2.tile([P, H], f16, tag="xg_sb")
                        nc.gpsimd.indirect_dma_start(
                            out=xg_sb[:], out_offset=None,
                            in_=xh_in[:, :],
                            in_offset=bass.IndirectOffsetOnAxis(
                                ap=disp_sb[:, ct:ct + 1], axis=0))
                        nc.sync.dma_start_transpose(
                            out=xgT[:, :, ct * P:(ct + 1) * P], in_=xg_sb[:])

                    cchunks = []
                    c0 = 0
                    while c0 < CAP:
                        cw = min(512, CAP - c0)
                        cchunks.append((c0, cw))
                        c0 += cw
                    for irt in range(IRT):
                        rgw = p2.tile([P, HC, P], f16, tag="rgw")
                        nc.sync.dma_start(
                            out=rgw[:],
                            in_=rg_in[:, irt * P:(irt + 1) * P].rearrange(
                                "(hc hp) i -> hp hc i", hp=P))
                        ruw = p2.tile([P, HC, P], f16, tag="ruw")
                        nc.sync.dma_start(
                            out=ruw[:],
                            in_=ru_in[:, irt * P:(irt + 1) * P].rearrange(
                                "(hc hp) i -> hp hc i", hp=P))
                        for (c0, cw) in cchunks:
                            ps_g = psa.tile([P, 512], f32, tag="ps_g", bufs=2)
                            ps_u = psa.tile([P, 512], f32, tag="ps_u", bufs=2)
                            for hc in range(HC):
                                nc.tensor.matmul(
                                    out=ps_g[:, :cw], lhsT=rgw[:, hc, :],
                                    rhs=xgT[:, hc, c0:c0 + cw],
                                    start=(hc == 0), stop=(hc == HC - 1))
                                nc.tensor.matmul(
                                    out=ps_u[:, :cw], lhsT=ruw[:, hc, :],
                                    rhs=xgT[:, hc, c0:c0 + cw],
                                    start=(hc == 0), stop=(hc == HC - 1))
                            sil = p2.tile([P, 512], f32, tag="sil")
                            nc.scalar.activation(out=sil[:, :cw],
                                                 in_=ps_g[:, :cw],
                                                 func=Act.Silu)
                            nc.vector.tensor_tensor(
                                out=hT[:, irt, c0:c0 + cw], in0=sil[:, :cw],
                                in1=ps_u[:, :cw], op=Alu.mult)

                # down: eout[c, h] = sum_ir h[c, ir] * rd[ir, h]
                with tc.tile_pool(name="p2psD", bufs=1, space="PSUM") as psd:
                    ct0 = 0
                    while ct0 < CT:
                        blk = min(4, CT - ct0)
                        ps_d = [[psd.tile([P, 512], f32, tag=f"d{i}{hn}",
                                          name=f"d{i}{hn}")
                                 for hn in range(NHT)] for i in range(blk)]
                        for ic in range(IRT):
                            rdw = p2.tile([P, H], f16, tag="rdw")
                            nc.sync.dma_start(
                                out=rdw[:], in_=rd_in[ic * P:(ic + 1) * P, :])
                            for i in range(blk):
                                for hn in range(NHT):
                                    nc.tensor.matmul(
                                        out=ps_d[i][hn][:],
                                        lhsT=hT[:, ic,
                                                (ct0 + i) * P:(ct0 + i + 1) * P],
                                        rhs=rdw[:, hn * 512:(hn + 1) * 512],
                                        start=(ic == 0), stop=(ic == IRT - 1))
                        for i in range(blk):
                            eo = p2.tile([P, H], f16, tag="eo")
                            for hn in range(NHT):
                                nc.vector.tensor_copy(
                                    out=eo[:, hn * 512:(hn + 1) * 512],
                                    in_=ps_d[i][hn][:])
                            nc.sync.dma_start(
                                out=eout_dram[(ct0 + i) * P:(ct0 + i + 1) * P, :],
                                in_=eo[:])
                        ct0 += blk

            # ------- P3: shared gu (9 supertiles) + down + combine + RS -----
            # Processing order alternates down-only (hsT spilled in phase A)
            # and full supertiles so per-chunk PE time (~27us avg) stays
            # above the CC core's ~24us serial ReduceScatter cadence.
            with tc.tile_pool(name="p3w", bufs=1) as p3w, \
                 tc.tile_pool(name="p3", bufs=2) as p3, \
                 tc.tile_pool(name="p3ps", bufs=1, space="PSUM") as p3ps:
                sdw = p3w.tile([P, ISHT, H], f16)
                nc.sync.dma_start(
                    out=sdw[:],
                    in_=sd_in[:].rearrange("(it ip) h -> ip it h", ip=P))
                sgw2 = p3w.tile([P, HC, ISH], f16)
                suw2 = p3w.tile([P, HC, ISH], f16)

                spilled = list(range(NSPILL))
                fulls = list(range(NSPILL, NST))
                order = []
                while spilled or fulls:
                    if spilled:
                        order.append(("D", spilled.pop(0)))
                    if fulls:
                        order.append(("F", fulls.pop(0)))

                GPF = 2  # entries of prefetch ahead
                gath_tiles = {}
                hs_tiles = {}
                xT_tiles = {}

                def prefetch(i):
                    if i >= len(order):
                        return
                    kind, st = order[i]
                    tiles = []
                    for ts in range(ST):
                        j = st * ST + ts
                        gath = p3.tile([P, H], f16, tag="gath", bufs=4 * GPF + 4)
                        nc.gpsimd.indirect_dma_start(
                            out=gath[:], out_offset=None,
                            in_=eout_dram[:, :],
                            in_offset=bass.IndirectOffsetOnAxis(
                                ap=slotg_i32[:, j:j + 1], axis=0))
                        tiles.append(gath)
                    gath_tiles[i] = tiles
                    if kind == "D":
                        hsT = p3.tile([P, ISHT, STT], f16, tag="hsS", bufs=3)
                        nc.sync.dma_start(out=hsT[:], in_=hs_dram[st])
                        hs_tiles[i] = hsT
                    else:
                        t0 = st * STT
                        xT = p3.tile([P, HC, STT], f16, tag="xTs", bufs=3)
                        nc.sync.dma_start(
                            out=xT[:],
                            in_=xht_in[:, t0:t0 + STT].rearrange(
                                "(hc hp) t -> hp hc t", hp=P))
                        xT_tiles[i] = xT

                prefetch(0)
                # gate/up weights reload after entry-0's tiles so the first
                # (down-only) supertile is not starved behind 4MB of weights
                nc.sync.dma_start(
                    out=sgw2[:], in_=sg_in[:].rearrange("(hc hp) i -> hp hc i", hp=P))
                nc.sync.dma_start(
                    out=suw2[:], in_=su_in[:].rearrange("(hc hp) i -> hp hc i", hp=P))
                prefetch(1)

                for i, (kind, st) in enumerate(order):
                    if kind == "D":
                        hsT = hs_tiles.pop(i)
                    else:
                        hsT = p3.tile([P, ISHT, STT], f16, tag="hsTf", bufs=2)
                        shared_gu(xT_tiles.pop(i), hsT, sgw2, suw2, p3, p3ps)
                    for ts in range(ST):
                        j = st * ST + ts
                        gath = gath_tiles[i][ts]
                        outt = p3.tile([P, H], f16, tag="outt")
                        for hn in range(NHT):
                            ps_d3 = p3ps.tile([P, 512], f32, tag="ps_d3",
                                              bufs=2)
                            for it in range(ISHT):
                                nc.tensor.matmul(
                                    out=ps_d3[:],
                                    lhsT=hsT[:, it, ts * P:(ts + 1) * P],
                                    rhs=sdw[:, it, hn * 512:(hn + 1) * 512],
                                    start=(it == 0), stop=(it == ISHT - 1))
                            nc.vector.scalar_tensor_tensor(
                                out=outt[:, hn * 512:(hn + 1) * 512],
                                in0=gath[:, hn * 512:(hn + 1) * 512],
                                scalar=wv_pm[:, j:j + 1],
                                in1=ps_d3[:],
                                op0=Alu.mult, op1=Alu.add)
                        r = j % JCH
                        nc.sync.dma_start(
                            out=chunk_dram[st][r * P:(r + 1) * P, :],
                            in_=outt[:])
                    del gath_tiles[i]
                    prefetch(i + GPF)
                    # one supertile == one RS chunk; issue in processing
                    # order (identical on every core, so collectives match)
                    nc.gpsimd.collective_compute(
                        "ReduceScatter", Alu.add,
                        replica_groups=replica_groups,
                        ins=[chunk_dram[st][:]],
                        outs=[rsout_dram[st][:]])
                    # fp16 -> f32 cast on the way out (SWDGE), one chunk late
                    # so the cast's RS-wait cannot delay the next gathers
                    if i > 0:
                        pst = order[i - 1][1]
                        nc.gpsimd.dma_start(out=o_out[pst],
                                            in_=rsout_dram[pst][:])
                lst = order[-1][1]
                nc.gpsimd.dma_start(out=o_out[lst], in_=rsout_dram[lst][:])

    nc.finalize()
    return nc


_NC_CACHE = {}


def _get_nc(key="full"):
    if key not in _NC_CACHE:
        _NC_CACHE[key] = build(FULL_CFG)
    return _NC_CACHE[key]


def make_in_maps(inputs, cfg=FULL_CFG):
    T, H, E, IR, IS, CAP, NCORES = (
        cfg[k] for k in ("T", "H", "E", "IR", "IS", "CAP", "NC"))
    J = T // P
    ISH = IS // NCORES
    f16 = np.float16
    x = np.ascontiguousarray(np.asarray(inputs["x"], np.float32).reshape(T, H))
    xh = x.astype(f16)
    xl = (x - xh.astype(np.float32)).astype(f16)
    xht = np.ascontiguousarray(xh.T)
    xlt = np.ascontiguousarray(xl.T)
    rw = np.asarray(inputs["router_w"], np.float32)
    rwt = np.ascontiguousarray(rw.T)
    rwh = rwt.astype(f16)
    rwl = (rwt - rwh.astype(np.float32)).astype(f16)
    rw16 = np.ascontiguousarray(np.concatenate([rwh, rwl], axis=1))
    rg = np.asarray(inputs["rg"], np.float32)
    ru = np.asarray(inputs["ru"], np.float32)
    rd = np.asarray(inputs["rd"], np.float32)
    sg = np.asarray(inputs["sg"], np.float32)
    su = np.asarray(inputs["su"], np.float32)
    sd = np.asarray(inputs["sd"], np.float32)

    tokidf = np.ascontiguousarray(
        np.arange(T, dtype=np.float32).reshape(J, P).T)      # [p, j] = 128j+p
    slotiota = np.ascontiguousarray(
        np.arange(CAP, dtype=np.float32).reshape(CAP // P, P).T)  # [p, ct]
    utri = np.triu(np.ones((J, J), np.float32), k=1)
    iotaf = np.tile(np.arange(E, dtype=np.float32), (P, J))  # [P, J*E]

    in_maps = []
    for i in range(NCORES):
        in_maps.append(dict(
            xh=xh, xht=xht, xlt=xlt, rw16=rw16,
            rg=np.ascontiguousarray(rg[i]).astype(f16),
            ru=np.ascontiguousarray(ru[i]).astype(f16),
            rd=np.ascontiguousarray(rd[i]).astype(f16),
            sg=np.ascontiguousarray(sg[:, i * ISH:(i + 1) * ISH]).astype(f16),
            su=np.ascontiguousarray(su[:, i * ISH:(i + 1) * ISH]).astype(f16),
            sd=np.ascontiguousarray(sd[i * ISH:(i + 1) * ISH, :]).astype(f16),
            eidf=np.full((P, J), float(i), np.float32),
            iotaf=iotaf, tokidf=tokidf, slotiota=slotiota, utri=utri,
        ))
    return in_maps


def assemble_output(results, cfg=FULL_CFG):
    T, H, NCORES = cfg["T"], cfg["H"], cfg["NC"]
    NCH = cfg.get("NCH", 16)
    rows_per = T // NCH // NCORES
    out = np.empty((T, H), np.float32)
    for i in range(NCORES):
        o = results[i]["o"]
        for k in range(NCH):
            base = (T // NCH) * k + rows_per * i
            out[base:base + rows_per] = o[k]
    return out


def kernel(**inputs):
    nc = _get_nc()
    in_maps = make_in_maps(inputs)
    core_ids = list(range(FULL_CFG["NC"]))
    last_err = None
    for _attempt in range(2):
        try:
            res = run_bass_kernel_spmd(nc, in_maps, core_ids, trace=False)
            break
        except Exception as e:  # transient device wedges: retry once
            last_err = e
    else:
        raise last_err
    out = assemble_output(res.results)
    B, S, H = 4, 2048, 1024
    return out.reshape(B, S, H)
